# revision 15
# baseline (speedup 1.0000x reference)
"""Bilinear interpolation kernel for Trainium2 (8 NeuronCores, SPMD).

Strategy (data-parallel over query points, image replicated per core):
  * Host shards coords [2, N] into 8 equal slices of N/8 points.
  * Device phase 1: build an interleaved "pairs" table in DRAM:
      pairs[y*W + x] = (img[y, x], img[y+1, x])   for y in [0, H-2]
    so the 4 bilinear corners for (y0, x0) are the 4 consecutive floats
    at pairs.flat[2*(y0*W + x0) : +4] -> one gather descriptor per point.
    The interleave is split across VectorE (even) and ScalarE (odd) in
    column strips so it overlaps the image loads and the main loop's
    coordinate prefetch.
  * Device phase 2: stream coord chunks, compute floor/clamp/weights/flat
    index on VectorE, gather 16B per point via indirect DMA (128 points
    per instruction — the HW max: one descriptor per partition), blend
    with a 2-level lerp, mask, stream out values (f32) + valid (u8).

  The indirect-DMA SWDGE issue cost (~1.1us per 128-descriptor
  instruction on the GpSimd engine) is the hard bottleneck; the design
  keeps GpSimd ~100% busy: gathers are gated only on the pairs table
  being stored, coords/phase-A run 4 chunks deep, gather buffers are
  3 deep, and the build interleave never touches GpSimd.
"""
import sys

sys.path.insert(0, "/opt/trn_rl_repo")

from contextlib import ExitStack

import numpy as np

import concourse.bass as bass
import concourse.mybir as mybir
from concourse.bass_utils import run_bass_kernel_spmd

H = W = 4096
N = 16777216
NCORES = 8
NPC = N // NCORES            # points per core (2_097_152)
PP = 128                     # partitions
K = 512                      # points per partition per chunk
CH = PP * K                  # chunk points (65536)
NS = NPC // CH               # chunks (32)
PROWS = (H - 1) * W          # pairs table rows
MAGIC = 8388608.0            # 2^23 fp32 round-to-int magic
CW = 2048                    # build column-strip width (image cols)

f32 = mybir.dt.float32
i32 = mybir.dt.int32
u8 = mybir.dt.uint8


def build_nc(npc=NPC, reps=1):
    ns = npc // CH
    nc = bass.Bass()
    x_d = nc.declare_dram_parameter("x", [H, W], f32, isOutput=False)
    xq_d = nc.declare_dram_parameter("xq", [npc], f32, isOutput=False)
    yq_d = nc.declare_dram_parameter("yq", [npc], f32, isOutput=False)
    val_d = nc.declare_dram_parameter("values", [npc], f32, isOutput=True)
    vld_d = nc.declare_dram_parameter("valid", [npc], u8, isOutput=True)
    pairs_d = nc.dram_tensor("pairs", [PROWS, 2], f32)

    es = ExitStack()
    # DMA-completion semaphores are parity/slot-indexed: a wait_ge on a DMA
    # sem is only sound if, at wait time, the issued DMAs counting toward it
    # are exactly the required ones (completions across queue rows can
    # reorder). Engine-incremented sems (p_cpv/p_cps/wdone/bdone) are safe.
    es2 = ExitStack()
    with (
        nc.semaphore("p_in0") as p_in0,    # build loads, even steps (32/step)
        nc.semaphore("p_in1") as p_in1,    # build loads, odd steps (32/step)
        nc.semaphore("p_cpv") as p_cpv,    # build: even interleave done (1/step)
        nc.semaphore("p_cps") as p_cps,    # build: odd interleave done (1/step)
        nc.semaphore("p_out0") as p_out0,  # build stores, even steps (16/step)
        nc.semaphore("p_out1") as p_out1,  # build stores, odd steps (16/step)
        nc.semaphore("wdone") as wdone,    # phase-A compute done (1/chunk)
        nc.semaphore("gdone") as gdone,    # gathers landed (16*K/chunk)
        nc.semaphore("bdone") as bdone,    # blend done (1/chunk)
        nc.Block() as block,
        es,
        es2,
    ):
        p_in = [p_in0, p_in1]
        p_out = [p_out0, p_out1]
        cin = [es2.enter_context(nc.semaphore(f"cin{i}")) for i in range(4)]
        ost = [es2.enter_context(nc.semaphore(f"ost{i}")) for i in range(3)]
        def sb(name, shape, dt):
            return es.enter_context(nc.sbuf_tensor(name, shape, dt))

        # build tiles (column strips, 2 bufs)
        ia = [sb(f"ia{b}", [PP, CW], f32) for b in range(2)]
        ib = [sb(f"ib{b}", [PP, CW], f32) for b in range(2)]
        pt = [sb(f"pt{b}", [PP, 2 * CW], f32) for b in range(2)]
        # main loop tiles
        txq = [sb(f"txq{b}", [PP, K], f32) for b in range(4)]
        tyq = [sb(f"tyq{b}", [PP, K], f32) for b in range(4)]
        tt = sb("tt", [PP, K], f32)
        tgt = sb("tgt", [PP, K], f32)
        tfx = sb("tfx", [PP, K], f32)
        tfy = sb("tfy", [PP, K], f32)
        txc = sb("txc", [PP, K], f32)
        tyc = sb("tyc", [PP, K], f32)
        tif = sb("tif", [PP, K], f32)
        tvf = [sb(f"tvf{b}", [PP, K], f32) for b in range(4)]
        tidx = [sb(f"tidx{b}", [PP, K], i32) for b in range(4)]
        ax1 = [sb(f"ax1{b}", [PP, K], f32) for b in range(4)]
        ay1 = [sb(f"ay1{b}", [PP, K], f32) for b in range(4)]
        gt_ = [sb(f"g{b}", [PP, 4 * K], f32) for b in range(3)]
        acc = [sb(f"acc{b}", [PP, K], f32) for b in range(3)]
        tvu = [sb(f"tvu{b}", [PP, K], u8) for b in range(3)]
        tw = sb("tw", [PP, K], f32)
        ts_ = sb("ts", [PP, K], f32)

        # build geometry: 32 row-bands x 4 column strips
        NBR = (H - 1 + PP - 1) // PP          # 32 (last band has 127 rows)
        NBC = W // CW                         # 4
        NBUILD = NBR * NBC

        def geom(s):
            r, c = divmod(s, NBC)
            r0 = r * PP
            return r0, min(PP, (H - 1) - r0), c * CW

        pairs_v = pairs_d[:].rearrange("(y x) two -> y x two", x=W)

        xq_t = xq_d[:].rearrange("(s p k) -> s p k", p=PP, k=K)
        yq_t = yq_d[:].rearrange("(s p k) -> s p k", p=PP, k=K)
        val_t = val_d[:].rearrange("(s p k) -> s p k", p=PP, k=K)
        vld_t = vld_d[:].rearrange("(s p k) -> s p k", p=PP, k=K)

        NG = reps * ns

        @block.sync
        def _(sync):
            # ---------- phase 1: pairs table build (overlapped w/ main) ----
            def bload(s):
                r0, nr, c0 = geom(s)
                b = s % 2
                sync.dma_start(
                    out=ia[b][:nr, :], in_=x_d[r0:r0 + nr, c0:c0 + CW]
                ).then_inc(p_in[s % 2], 16)
                sync.dma_start(
                    out=ib[b][:nr, :], in_=x_d[r0 + 1:r0 + 1 + nr, c0:c0 + CW]
                ).then_inc(p_in[s % 2], 16)

            bload(0)
            if NBUILD > 1:
                bload(1)
            for s in range(NBUILD):
                r0, nr, c0 = geom(s)
                b = s % 2
                sync.wait_ge(p_cpv, s + 1)
                sync.wait_ge(p_cps, s + 1)
                sync.dma_start(
                    out=pairs_v[r0:r0 + nr, c0:c0 + CW, :]
                    .rearrange("y x two -> y (x two)"),
                    in_=pt[b][:nr, :],
                ).then_inc(p_out[s % 2], 16)
                if s + 2 < NBUILD:
                    bload(s + 2)   # ia/ib buf s consumed (p_cpv/p_cps waited)

            # ---------- phase 2: coords in / results out (interleaved) -----
            def ostore(c):
                sync.dma_start(
                    out=val_t[c % ns], in_=acc[c % 3][:]
                ).then_inc(ost[c % 3], 16)
                sync.dma_start(
                    out=vld_t[c % ns], in_=tvu[c % 3][:]
                ).then_inc(ost[c % 3], 16)

            for g in range(NG):
                s = g % ns
                if g >= 4:
                    sync.wait_ge(wdone, g - 3)   # coord buf g-4 consumed
                sync.dma_start(out=txq[g % 4][:], in_=xq_t[s]).then_inc(cin[g % 4], 16)
                sync.dma_start(out=tyq[g % 4][:], in_=yq_t[s]).then_inc(cin[g % 4], 16)
                if g >= 3:
                    sync.wait_ge(bdone, g - 2)
                    ostore(g - 3)
            for g in range(max(0, NG - 3), NG):
                sync.wait_ge(bdone, g + 1)
                ostore(g)
            for r in range(3):
                cnt = len(range(r, NG, 3))
                if cnt:
                    sync.wait_ge(ost[r], 32 * cnt)

        @block.scalar
        def _(scalar):
            # build: odd interleave half on ACT
            for s in range(NBUILD):
                r0, nr, c0 = geom(s)
                b = s % 2
                scalar.wait_ge(p_in[s % 2], 32 * (s // 2 + 1))
                if s >= 2:
                    scalar.wait_ge(p_out[s % 2], 16 * (s // 2))  # pt s-2 stored
                scalar.copy(pt[b][:nr, 1:2 * CW:2], ib[b][:nr, :]).then_inc(p_cps, 1)

        @block.vector
        def _(vector):
            A = mybir.AluOpType
            # build: even interleave half on DVE
            for s in range(NBUILD):
                r0, nr, c0 = geom(s)
                b = s % 2
                vector.wait_ge(p_in[s % 2], 32 * (s // 2 + 1))
                if s >= 2:
                    vector.wait_ge(p_out[s % 2], 16 * (s // 2))
                vector.tensor_copy(pt[b][:nr, 0:2 * CW:2], ia[b][:nr, :]).then_inc(p_cpv, 1)

            # ---------- phase 2 ----------
            def phase_a(g):
                b = g % 4
                vector.wait_ge(cin[g % 4], 32 * (g // 4 + 1))
                if g >= 4:
                    # tidx buf g-4 fully consumed once chunk g-4 gathers landed
                    vector.wait_ge(gdone, 16 * K * (g - 3))
                xq, yq = txq[b][:], tyq[b][:]
                # floor(xq) -> tfx
                vector.tensor_scalar_add(tt[:], xq, MAGIC)
                vector.tensor_scalar_sub(tt[:], tt[:], MAGIC)
                vector.tensor_tensor(out=tgt[:], in0=tt[:], in1=xq, op=A.is_gt)
                vector.tensor_tensor(out=tfx[:], in0=tt[:], in1=tgt[:], op=A.subtract)
                # floor(yq) -> tfy
                vector.tensor_scalar_add(tt[:], yq, MAGIC)
                vector.tensor_scalar_sub(tt[:], tt[:], MAGIC)
                vector.tensor_tensor(out=tgt[:], in0=tt[:], in1=yq, op=A.is_gt)
                vector.tensor_tensor(out=tfy[:], in0=tt[:], in1=tgt[:], op=A.subtract)
                # clamps + validity
                vector.tensor_scalar(txc[:], tfx[:], 0.0, float(W - 2), A.max, A.min)
                vector.tensor_scalar(tyc[:], tfy[:], 0.0, float(H - 2), A.max, A.min)
                vector.tensor_tensor(out=tt[:], in0=txc[:], in1=tfx[:], op=A.is_equal)
                vector.tensor_tensor(out=tgt[:], in0=tyc[:], in1=tfy[:], op=A.is_equal)
                vector.tensor_tensor(out=tvf[b][:], in0=tt[:], in1=tgt[:], op=A.mult)
                # flat index = yc*W + xc (exact in fp32), cast int32
                vector.tensor_scalar_mul(tif[:], tyc[:], float(W))
                vector.tensor_tensor(out=tif[:], in0=tif[:], in1=txc[:], op=A.add)
                vector.tensor_copy(tidx[b][:], tif[:])
                # weights
                vector.tensor_tensor(out=ax1[b][:], in0=xq, in1=tfx[:], op=A.subtract)
                vector.tensor_tensor(out=ay1[b][:], in0=yq, in1=tfy[:], op=A.subtract) \
                    .then_inc(wdone, 1)

            def phase_b(g):
                b4 = g % 4
                b3 = g % 3
                vector.wait_ge(gdone, 16 * K * (g + 1))
                if g >= 3:
                    # acc/tvu slot g%3 free once chunk g-3 stores landed
                    vector.wait_ge(ost[g % 3], 32 * ((g - 3) // 3 + 1))
                G = gt_[b3][:]
                v00 = G[:, 0:4 * K:4]
                v01 = G[:, 1:4 * K:4]
                v10 = G[:, 2:4 * K:4]
                v11 = G[:, 3:4 * K:4]
                # top = v00 + ax1*(v10-v00); bot = v01 + ax1*(v11-v01)
                # out = (top + ay1*(bot-top)) * valid
                vector.tensor_tensor(out=tw[:], in0=v10, in1=v00, op=A.subtract)
                vector.tensor_tensor(out=tw[:], in0=tw[:], in1=ax1[b4][:], op=A.mult)
                vector.tensor_tensor(out=ts_[:], in0=v00, in1=tw[:], op=A.add)
                vector.tensor_tensor(out=tw[:], in0=v11, in1=v01, op=A.subtract)
                vector.tensor_tensor(out=tw[:], in0=tw[:], in1=ax1[b4][:], op=A.mult)
                vector.tensor_tensor(out=acc[b3][:], in0=v01, in1=tw[:], op=A.add)
                vector.tensor_tensor(out=acc[b3][:], in0=acc[b3][:], in1=ts_[:], op=A.subtract)
                vector.tensor_tensor(out=acc[b3][:], in0=acc[b3][:], in1=ay1[b4][:], op=A.mult)
                vector.tensor_tensor(out=acc[b3][:], in0=acc[b3][:], in1=ts_[:], op=A.add)
                vector.tensor_tensor(out=acc[b3][:], in0=acc[b3][:], in1=tvf[b4][:], op=A.mult)
                vector.tensor_copy(tvu[b3][:], tvf[b4][:]).then_inc(bdone, 1)

            for g in range(min(4, NG)):
                phase_a(g)
            for g in range(NG):
                phase_b(g)
                if g + 4 < NG:
                    phase_a(g + 4)

        @block.gpsimd
        def _(gpsimd):
            gpsimd.wait_ge(p_out0, 16 * ((NBUILD + 1) // 2))  # pairs stored
            gpsimd.wait_ge(p_out1, 16 * (NBUILD // 2))
            for g in range(NG):
                gpsimd.wait_ge(wdone, g + 1)
                if g >= 2:
                    gpsimd.wait_ge(bdone, g - 1)   # G buf free (2-deep throttle)
                b4 = g % 4
                b3 = g % 3
                for j in range(K):
                    gpsimd.indirect_dma_start(
                        out=gt_[b3][:, 4 * j:4 * j + 4],
                        out_offset=None,
                        in_=pairs_d[:],
                        in_offset=bass.IndirectOffsetOnAxis(ap=tidx[b4][:, j:j + 1], axis=0),
                    ).then_inc(gdone, 16)

    return nc


_nc_cache = None


def kernel(x: np.ndarray, coords: np.ndarray):
    global _nc_cache
    if _nc_cache is None:
        _nc_cache = build_nc()
    nc = _nc_cache

    x = np.ascontiguousarray(np.asarray(x), dtype=np.float32)
    coords = np.asarray(coords, dtype=np.float32)
    in_maps = []
    for c in range(NCORES):
        sl = slice(c * NPC, (c + 1) * NPC)
        in_maps.append({
            "x": x,
            "xq": np.ascontiguousarray(coords[0, sl]),
            "yq": np.ascontiguousarray(coords[1, sl]),
        })
    res = run_bass_kernel_spmd(nc, in_maps, list(range(NCORES)))
    values = np.concatenate([res.results[c]["values"] for c in range(NCORES)])
    valid = np.concatenate([res.results[c]["valid"] for c in range(NCORES)]).astype(bool)
    return values, valid
